# revision 1
# baseline (speedup 1.0000x reference)
"""Causal self-attention (B=4, T=2048, C=1024, H=16) on 8 TRN2 NeuronCores.

Sharding: tensor-parallel over heads. Each core owns 2 heads:
  - computes its 384-column slice of the QKV projection (q|k|v, 128 cols each)
    directly in transposed layout qkvT = w_slice.T @ xT (x is host-pre-transposed),
  - runs causal attention for its 8 (batch, head) pairs in scores-transposed
    form sT = kT.T @ qT so that softmax(p) feeds the p@v matmul with no PE
    transposes (pT == sT); the softmax normalizer Z is accumulated by an
    appended ones-column in the v stationary operand,
  - normalizes y by 1/Z and multiplies by its 128-row slice of w_proj,
    producing a partial [8192, 1024] output.
Host sums the 8 partials and adds b_proj.

All matmuls run in bf16 (fp32 PSUM accumulation; Z kept in fp32). QKV of
batch b+1 is emission-interleaved into attention of batch b so the PE stays
dense/warm while ScalarE runs exp, and the two heads' score matmuls alternate
PE row groups so their LDWEIGHTS overlap.
"""

import numpy as np

import concourse.bass as bass
import concourse.mybir as mybir
import concourse.tile as tile
from concourse import bacc
from concourse.bass_utils import run_bass_kernel_spmd
from concourse.masks import make_identity

B, T, C, H, D = 4, 2048, 1024, 16, 64
NCORES = 8
HPC = H // NCORES          # heads per core = 2
SH = HPC * D               # 128: shard width of each of q/k/v
R = B * T                  # 8192 rows
KC = C // 128              # 8 contraction chunks
NT = R // 512              # 16 row-groups of 512
QG = T // 512              # 4 query groups per (batch, head) pair
CPB = T // 128             # 16 key chunks per batch
f32 = mybir.dt.float32
f32r = mybir.dt.float32r
bf16 = mybir.dt.bfloat16
EXP = mybir.ActivationFunctionType.Exp

TRACE = False
TRACE_KWARGS = {}
LAST_RESULT = None
_NC_CACHE = None


def _emit(tc, xT, w_s, b_s, wp_s, out):
    nc = tc.nc
    # DRAM bounce for transposing softmax-denominator rows into column layout
    zdram = nc.dram_tensor("zdram", [2 * B * QG * 512], f32, kind="Internal").ap()

    with (
        tc.tile_pool(name="const", bufs=1) as constp,
        tc.tile_pool(name="qkv", bufs=2) as qkvp,
        tc.tile_pool(name="xt", bufs=12) as xtp,
        tc.tile_pool(name="vstage", bufs=3) as vstagep,
        tc.tile_pool(name="pt", bufs=6) as ptp,
        tc.tile_pool(name="yu", bufs=2) as yup,
        tc.tile_pool(name="zcol", bufs=2) as zcolp,
        tc.tile_pool(name="ztmp", bufs=4) as ztmpp,
        tc.tile_pool(name="ostage", bufs=6) as ostagep,
        tc.tile_pool(name="ps_s", bufs=2, space="PSUM") as ps_s,
        tc.tile_pool(name="ps_o", bufs=2, space="PSUM") as ps_o,
        tc.tile_pool(name="aux", bufs=2, space="PSUM") as auxp,
    ):
        ident = constp.tile([128, 128], bf16)
        make_identity(nc, ident[:])
        ones_f = constp.tile([128, 128], bf16)
        nc.gpsimd.memset(ones_f[:], 1.0)
        w_sb = constp.tile([128, KC, 3 * SH], bf16)
        nc.sync.dma_start(w_sb[:], w_s.rearrange("(kc p) m -> p kc m", p=128))
        b_sb = constp.tile([128, 3], f32)
        nc.sync.dma_start(b_sb[:], b_s.rearrange("(m p) -> p m", p=128))
        wp_sb = constp.tile([128, C], bf16)
        nc.sync.dma_start(wp_sb[:], wp_s)

        # per-batch persistent tiles (double-buffered across the pipeline)
        qTs, kTs, vaugs = {}, {}, {}

        def qkv_units(b):
            """Emit QKV projection for batch b as 4 row-group units."""
            qT = qkvp.tile([128, T], bf16, name=f"qT_{b}", tag="qT")
            kT = qkvp.tile([128, T], bf16, name=f"kT_{b}", tag="kT")
            vaug = qkvp.tile([128, 2 * CPB * 65], bf16, name=f"vaug_{b}", tag="vaug")
            qTs[b], kTs[b], vaugs[b] = qT, kT, vaug

            def ones_unit():
                nc.vector.tensor_copy(
                    vaug[:].rearrange("p (blk c) -> p blk c", c=65)[:, :, 64:65],
                    ones_f[:, 0 : 2 * CPB].unsqueeze(2),
                )

            def ngroup(nl):
                def unit():
                    n = 4 * b + nl
                    xts = []
                    for k in range(KC):
                        xt = xtp.tile([128, 512], bf16, name=f"xt_{n}_{k}", tag="xt")
                        nc.sync.dma_start(
                            xt[:], xT[128 * k : 128 * (k + 1), 512 * n : 512 * (n + 1)]
                        )
                        xts.append(xt)
                    nsl = slice(512 * nl, 512 * (nl + 1))
                    for m in range(3):
                        ps = auxp.tile([128, 512], f32, name=f"ps_{n}_{m}", tag="aux")
                        for k in range(KC):
                            nc.tensor.matmul(
                                ps[:],
                                w_sb[:, k, 128 * m : 128 * (m + 1)],
                                xts[k][:],
                                start=(k == 0),
                                stop=(k == KC - 1),
                            )
                        if m == 0:
                            nc.scalar.add(qT[:, nsl], ps[:], b_sb[:, 0:1])
                        elif m == 1:
                            nc.scalar.add(kT[:, nsl], ps[:], b_sb[:, 1:2])
                        else:
                            vst = vstagep.tile(
                                [128, 512], bf16, name=f"vst_{n}", tag="vst"
                            )
                            nc.scalar.add(vst[:], ps[:], b_sb[:, 2:3])
                            for j in range(4):
                                c_local = nl * 4 + j
                                tp = auxp.tile(
                                    [128, 128], bf16, name=f"tp_{n}_{j}", tag="aux"
                                )
                                nc.tensor.transpose(
                                    tp[:], vst[:, 128 * j : 128 * (j + 1)], ident[:]
                                )
                                for h in range(2):
                                    off = (h * CPB + c_local) * 65
                                    nc.vector.tensor_copy(
                                        vaug[:, off : off + 64],
                                        tp[:, 64 * h : 64 * h + 64],
                                    )

                return unit

            units = [ngroup(nl) for nl in range(4)]
            units[0] = (lambda u: (lambda: (ones_unit(), u())))(units[0])
            return units

        def att_units(b):
            """Attention + projection for batch b; heads interleaved per kc2
            so s-matmul LDWEIGHTS (row groups 0-63 vs 64-127) overlap."""
            qT, kT, vaug = qTs[b], kTs[b], vaugs[b]
            yu = yup.tile([128, T], bf16, name=f"yu_{b}", tag="yu")
            # zcol[p, h*16 + rt] = softmax denom Z for (head h, t = 128*rt + p)
            zcol = zcolp.tile([128, 32], f32, name=f"zcol_{b}", tag="zcol")
            units = []

            def g_unit(g):
                nkc = 4 * g + 4
                ots = [
                    ps_o.tile([65, 512], f32, name=f"ot_{b}_{h}_{g}", tag="ot")
                    for h in range(2)
                ]
                for kc2 in range(nkc // 2):
                    sps = [
                        ps_s.tile(
                            [128, 1024], f32, name=f"sp_{b}_{h}_{g}_{kc2}", tag="sp"
                        )
                        for h in range(2)
                    ]
                    for half in range(2):
                        kc = 2 * kc2 + half
                        for h in range(2):
                            hsl = slice(64 * h, 64 * h + 64)
                            nc.tensor.matmul(
                                sps[h][:, 512 * half : 512 * (half + 1)],
                                kT[hsl, 128 * kc : 128 * (kc + 1)],
                                qT[hsl, 512 * g : 512 * (g + 1)],
                                start=True,
                                stop=True,
                            )
                    pts = []
                    for h in range(2):
                        pt = ptp.tile(
                            [128, 1024], bf16, name=f"pt_{b}_{h}_{g}_{kc2}", tag="pt"
                        )
                        nc.scalar.activation(pt[:], sps[h][:], EXP, scale=0.125)
                        pts.append(pt)
                    for half in range(2):
                        kc = 2 * kc2 + half
                        j = kc - 4 * g
                        if j >= 0:
                            for h in range(2):
                                # keep iff f >= 128*j + p  (tk <= tq)
                                nc.gpsimd.affine_select(
                                    out=pts[h][:, 512 * half : 512 * (half + 1)],
                                    in_=pts[h][:, 512 * half : 512 * (half + 1)],
                                    compare_op=mybir.AluOpType.is_ge,
                                    fill=0.0,
                                    base=-(128 * j),
                                    channel_multiplier=-1,
                                    pattern=[[1, 512]],
                                )
                    for half in range(2):
                        kc = 2 * kc2 + half
                        for h in range(2):
                            off = (h * CPB + kc) * 65
                            nc.tensor.matmul(
                                ots[h][:],
                                vaug[:, off : off + 65],
                                pts[h][:, 512 * half : 512 * (half + 1)],
                                start=(kc == 0),
                                stop=(kc == nkc - 1),
                            )
                gsl = slice(512 * g, 512 * (g + 1))
                for h in range(2):
                    pair = b * 2 + h
                    hsl = slice(64 * h, 64 * h + 64)
                    stage = ztmpp.tile(
                        [65, 512], bf16, name=f"stage_{pair}_{g}", tag="stage"
                    )
                    nc.vector.tensor_copy(stage[0:64, :], ots[h][0:64, :])
                    zst = ztmpp.tile([65, 512], f32, name=f"zst_{pair}_{g}", tag="zst")
                    nc.vector.tensor_copy(zst[64:65, :], ots[h][64:65, :])
                    # y rows -> yu at this head's partition block (DMA may shift
                    # partitions; engines may not)
                    nc.sync.dma_start(yu[hsl, gsl], stage[0:64, :])
                    # transpose Z row [1, 512] -> zcol [128, 4] (t = 128*tt + p)
                    zd = zdram[(pair * QG + g) * 512 : (pair * QG + g + 1) * 512]
                    nc.sync.dma_start(zd, zst[64:65, :])
                    zc = slice(16 * h + 4 * g, 16 * h + 4 * (g + 1))
                    nc.sync.dma_start(
                        zcol[:, zc], zd.rearrange("(tt p) -> p tt", p=128)
                    )
                    nc.vector.reciprocal(zcol[:, zc], zcol[:, zc])

            def proj_unit(rt):
                def unit():
                    rsl = slice(128 * rt, 128 * (rt + 1))
                    r0 = b * T + 128 * rt
                    for jn in range(2):
                        nsl = slice(512 * jn, 512 * (jn + 1))
                        pp0 = auxp.tile(
                            [128, 512], f32, name=f"pp0_{b}_{rt}_{jn}", tag="aux"
                        )
                        pp1 = auxp.tile(
                            [128, 512], f32, name=f"pp1_{b}_{rt}_{jn}", tag="aux"
                        )
                        nc.tensor.matmul(
                            pp0[:], yu[0:64, rsl], wp_sb[0:64, nsl],
                            start=True, stop=True,
                        )
                        nc.tensor.matmul(
                            pp1[:], yu[64:128, rsl], wp_sb[64:128, nsl],
                            start=True, stop=True,
                        )
                        ost = ostagep.tile(
                            [128, 512], f32, name=f"ost_{b}_{rt}_{jn}", tag="ost"
                        )
                        nc.vector.tensor_scalar_mul(
                            ost[:], pp0[:], zcol[:, rt : rt + 1]
                        )
                        nc.vector.scalar_tensor_tensor(
                            ost[:],
                            pp1[:],
                            zcol[:, 16 + rt : 16 + rt + 1],
                            ost[:],
                            op0=mybir.AluOpType.mult,
                            op1=mybir.AluOpType.add,
                        )
                        nc.sync.dma_start(out[r0 : r0 + 128, nsl], ost[:])

                return unit

            for g in range(QG):
                units.append((lambda g: (lambda: g_unit(g)))(g))
                for rt in range(4 * g, 4 * (g + 1)):
                    units.append(proj_unit(rt))
            return units

        # ---- software pipeline: QKV(b+1) interleaved into attention(b) ----
        for u in qkv_units(0):
            u()
        for b in range(B):
            au = att_units(b)
            qu = qkv_units(b + 1) if b + 1 < B else []
            stride = max(1, len(au) // (len(qu) + 1))
            qi = 0
            for i, u in enumerate(au):
                u()
                if qi < len(qu) and (i + 1) % stride == 0:
                    qu[qi]()
                    qi += 1
            while qi < len(qu):
                qu[qi]()
                qi += 1


def build_nc():
    global _NC_CACHE
    if _NC_CACHE is not None:
        return _NC_CACHE
    nc = bacc.Bacc("TRN2", target_bir_lowering=False, debug=False)
    xT = nc.dram_tensor("xT", [C, R], bf16, kind="ExternalInput").ap()
    w_s = nc.dram_tensor("w_s", [C, 3 * SH], bf16, kind="ExternalInput").ap()
    b_s = nc.dram_tensor("b_s", [3 * SH], f32, kind="ExternalInput").ap()
    wp_s = nc.dram_tensor("wp_s", [SH, C], bf16, kind="ExternalInput").ap()
    out = nc.dram_tensor("out", [R, C], f32, kind="ExternalOutput").ap()
    with tile.TileContext(nc) as tc:
        _emit(tc, xT, w_s, b_s, wp_s, out)
    nc.compile()
    _NC_CACHE = nc
    return nc


def kernel(x, w_attn, b_attn, w_proj, b_proj):
    global LAST_RESULT
    x = np.asarray(x, dtype=np.float32)
    w_attn = np.asarray(w_attn, dtype=np.float32)
    b_attn = np.asarray(b_attn, dtype=np.float32)
    w_proj = np.asarray(w_proj, dtype=np.float32)
    b_proj = np.asarray(b_proj, dtype=np.float32)

    import ml_dtypes

    xTh = np.ascontiguousarray(x.reshape(R, C).T.astype(ml_dtypes.bfloat16))  # [C, R]
    in_maps = []
    for c in range(NCORES):
        csl = slice(SH * c, SH * (c + 1))
        w_s = np.ascontiguousarray(
            np.concatenate(
                [w_attn[:, csl], w_attn[:, C:][:, csl], w_attn[:, 2 * C :][:, csl]],
                axis=1,
            )
        )
        b_s = np.ascontiguousarray(
            np.concatenate([b_attn[csl], b_attn[C:][csl], b_attn[2 * C :][csl]])
        )
        wp_s = np.ascontiguousarray(w_proj[csl, :].astype(ml_dtypes.bfloat16))
        w_s = w_s.astype(ml_dtypes.bfloat16)
        in_maps.append({"xT": xTh, "w_s": w_s, "b_s": b_s, "wp_s": wp_s})

    nc = build_nc()
    res = run_bass_kernel_spmd(
        nc,
        in_maps,
        core_ids=list(range(NCORES)),
        trace=TRACE,
        **TRACE_KWARGS,
    )
    LAST_RESULT = res
    acc = np.zeros((R, C), dtype=np.float64)
    for c in range(NCORES):
        acc += res.results[c]["out"]
    out = (acc + b_proj.astype(np.float64)).astype(np.float32)
    return out.reshape(B, T, C)

